# revision 4
# baseline (speedup 1.0000x reference)
"""Trainium2 Bass kernel for nn_Diffuser_78331613544465.

Math (per graph b of B=8, N=1024):
    A   = adj (mask is all-ones in the graded setup; general mask handled host-side)
    P   = A / max(rowsum(A), 1)
    out[i,j,:] = relu([I, P, P2, P4][i,j,:] @ w1 + b1) @ w2 + b2   (P2=P@P, P4=P2@P2)

Device strategy: data-parallel over B — one graph per NeuronCore (8 cores).
On-chip work happens in the TRANSPOSED domain (Q = P^T).  Because A is
symmetric, both P = D^-1 A (row scale) and Q = A D^-1 (col scale) come from
cheap elementwise scalings of A, and the power chain needs NO PE transposes:
    Q2 = P^T Q,  P2 = Q^T P,  Q4 = P2^T Q2      (matmul(lhsT=X, rhs=Y) = X^T Y)
The edge MLP runs with j on partitions / i on the moving dim; layer 1 is a
K=24 block-diagonal matmul over interleaved (j,s) rows, layer 2 a K=128
block-diagonal matmul.  The [16j x 8o, i] PSUM result is evacuated as-is
(fp16) and the HOST un-transposes — this keeps every output-DMA descriptor a
full 2KB partition line instead of 128B chunks (4x fewer SDMA descriptors).

HAM notes: a warm-up matmul stream covers the input-DMA phase so the PE
clock-gate reaches 8/8 before the power chain, and the per-al DRAM spills +
early ring prefetch remove the PE idle gap at the squares->MLP transition
that re-throttled the baseline to 1.2 GHz for its entire MLP phase.

kernel(**inputs) takes FULL inputs, shards over 8 cores, returns FULL output.
"""

import os
import numpy as np

B, N, P = 8, 1024, 128
HID, HEADS, NSTACK = 16, 8, 4
NT = N // P          # 8 row-tiles
JBLK = 8             # j rows per MLP block
NJB = N // JBLK      # 128 j-blocks
IC = 512             # i-chunk (matmul free dim)
NIC = N // IC        # 2
NSEG = 8             # ilbig ring segments (2 groups of 4 j-blocks)
WARMUP_PER_TILE = 5  # PE warm-up matmuls per adj tile DMA

# relu-on-vector j-blocks (~24 of 128): balances Scalar (relu) against
# Vector (relu share + all po evacuations)
VRELU = frozenset(jb for jb in range(NJB) if jb % 16 in (5, 10, 15))

_CACHE = {}
LAST_RESULTS = None


def _emit(nc, tc, ctx, mm_dt):
    from concourse import mybir

    f32 = mybir.dt.float32
    add = mybir.AluOpType.add
    amax = mybir.AluOpType.max
    mult = mybir.AluOpType.mult
    relu_fn = mybir.ActivationFunctionType.Relu

    adj = nc.declare_dram_parameter("adj", [N, N], f32, isOutput=False)
    w1blk_d = nc.declare_dram_parameter("w1blk", [3 * JBLK, P], mm_dt, isOutput=False)
    w1diag_d = nc.declare_dram_parameter("w1diag", [JBLK, P], mm_dt, isOutput=False)
    w2blk_d = nc.declare_dram_parameter("w2blk", [P, JBLK * HEADS], mm_dt, isOutput=False)
    b1rep_d = nc.declare_dram_parameter("b1rep", [P, 1], f32, isOutput=False)
    i8_d = nc.declare_dram_parameter("i8", [JBLK, JBLK], mm_dt, isOutput=False)
    idn32_d = nc.declare_dram_parameter("idn32", [P, P], f32, isOutput=False)
    # device-natural output: [jb-pair, (16j x 8o) partition, i] in fp16;
    # host transposes to [i, j, o] and casts to f32
    out = nc.declare_dram_parameter("out", [NJB // 2, P, N], mm_dt, isOutput=True)

    from contextlib import ExitStack

    small = ctx.enter_context(tc.tile_pool(name="small", bufs=1))
    big = ctx.enter_context(tc.tile_pool(name="big", bufs=1))
    rpool = ctx.enter_context(tc.tile_pool(name="rpool", bufs=3))
    ppool = ctx.enter_context(tc.tile_pool(name="ppool", bufs=3))
    dram = ctx.enter_context(tc.tile_pool(name="dram", bufs=1, space="DRAM"))
    ph14 = ExitStack()
    mm_ps = ph14.enter_context(tc.tile_pool(name="mm_ps", bufs=2, space="PSUM"))

    # persistent matrices, one [128, 1024] tile per 128-row band
    Af = [big.tile([P, N], f32, name=f"Af{t}", tag=f"Af{t}") for t in range(NT)]
    Pf = [big.tile([P, N], mm_dt, name=f"Pf{t}", tag=f"Pf{t}") for t in range(NT)]
    Qf = [big.tile([P, N], mm_dt, name=f"Qf{t}", tag=f"Qf{t}") for t in range(NT)]
    Q2f = [big.tile([P, N], mm_dt, name=f"Q2f{t}", tag=f"Q2f{t}") for t in range(NT)]
    P2f = [big.tile([P, N], mm_dt, name=f"P2f{t}", tag=f"P2f{t}") for t in range(NT)]
    Q4f = [big.tile([P, N], mm_dt, name=f"Q4f{t}", tag=f"Q4f{t}") for t in range(NT)]
    invrep = big.tile([P, N], f32, tag="invrep")
    ilbig = big.tile([3 * JBLK, NSEG * N], mm_dt, tag="ilbig")
    # DRAM spills of Q/Q2/Q4 (per 128-row band so the MLP ring can start
    # while later bands are still being computed); the interleaved ring
    # loads need row-hopping APs only DRAM sources allow
    Qd = [dram.tile([P, N], mm_dt, name=f"Qd{t}", tag=f"Qd{t}") for t in range(NT)]
    Q2d = [dram.tile([P, N], mm_dt, name=f"Q2d{t}", tag=f"Q2d{t}") for t in range(NT)]
    Q4d = [dram.tile([P, N], mm_dt, name=f"Q4d{t}", tag=f"Q4d{t}") for t in range(NT)]

    # ---- constants / weights (host-prepared; one DMA each) -----------------
    idn32 = small.tile([P, P], f32, tag="idn32")
    nc.gpsimd.dma_start(idn32[:], idn32_d[:])
    i8 = small.tile([JBLK, JBLK], mm_dt, tag="i8")
    nc.gpsimd.dma_start(i8[:], i8_d[:])
    ones1 = small.tile([1, P], f32, tag="ones1")
    nc.vector.memset(ones1[:], 1.0)
    w1blk = small.tile([3 * JBLK, P], mm_dt, tag="w1blk")
    nc.gpsimd.dma_start(w1blk[:], w1blk_d[:])
    w1diag = small.tile([JBLK, P], mm_dt, tag="w1diag")
    nc.gpsimd.dma_start(w1diag[:], w1diag_d[:])
    w2blk = small.tile([P, JBLK * HEADS], mm_dt, tag="w2blk")
    nc.gpsimd.dma_start(w2blk[:], w2blk_d[:])
    b1rep = small.tile([P, 1], f32, tag="b1rep")
    nc.gpsimd.dma_start(b1rep[:], b1rep_d[:])

    # ---- phase 1: load adj, deg -> invdeg, P = rowscale(A) ------------------
    # warm-up matmul stream on the PE keeps the HAM clock-gate busy through
    # the DMA-bound phase so the power chain starts at 2.4 GHz
    wm = mm_ps.tile([P, IC], f32, tag="wm")
    invcol = small.tile([P, NT], f32, tag="invcol")
    for t in range(NT):
        nc.gpsimd.dma_start(Af[t][:], adj[P * t:P * (t + 1), :])
        for k in range(WARMUP_PER_TILE):
            nc.tensor.matmul(
                wm[:], ones1[:], Af[t][0:1, IC * (k % NIC):IC * (k % NIC + 1)],
                start=True, stop=True,
            )
        deg = small.tile([P, 1], f32, tag=f"deg{t}")
        nc.vector.tensor_reduce(
            deg[:], Af[t][:], axis=mybir.AxisListType.X, op=add,
        )
        degc = small.tile([P, 1], f32, tag=f"degc{t}")
        nc.vector.tensor_scalar_max(degc[:], deg[:], 1.0)
        nc.vector.reciprocal(invcol[:, t:t + 1], degc[:])
        # P = A * invdeg[row]  (per-partition scale on the scalar engine)
        nc.scalar.mul(Pf[t][:], Af[t][:], invcol[:, t:t + 1])
    wscr = small.tile([P, 1], f32, tag="wscr")
    nc.scalar.copy(wscr[:], wm[:, 0:1])  # warm-up chain needs a reader

    # invrep[p, c] = invdeg(row c) for all p  (transpose + broadcast via PE)
    invrow = small.tile([1, N], f32, tag="invrow")
    for t in range(NT):
        ptp = mm_ps.tile([P, P], f32, tag="pt")
        nc.tensor.transpose(ptp[0:1, :], invcol[:, t:t + 1], idn32[:])
        nc.scalar.copy(invrow[0:1, P * t:P * (t + 1)], ptp[0:1, :])
    for half in range(2):
        pb = mm_ps.tile([P, IC], f32, tag="mm")
        for k in range(4):
            c = 4 * half + k
            nc.tensor.matmul(
                pb[:, P * k:P * (k + 1)], ones1[:], invrow[0:1, P * c:P * (c + 1)],
                start=True, stop=True,
            )
        nc.scalar.copy(invrep[:, IC * half:IC * (half + 1)], pb[:])

    # Q = A * invdeg[col]; spill each band for the MLP ring
    for t in range(NT):
        eng = nc.vector if t % 2 == 0 else nc.gpsimd
        eng.tensor_tensor(Qf[t][:], Af[t][:], invrep[:], op=mult)
        nc.sync.dma_start(Qd[t][:], Qf[t][:])

    # ---- power chain: X2[r,c] = sum_k L[k,r] R[k,c]  (no transposes) --------
    def square(lhs, rhs, dst, spill):
        for al in range(NT):
            for be in range(NIC):
                mm = mm_ps.tile([P, IC], f32, tag="mm")
                for g in range(NT):
                    nc.tensor.matmul(
                        mm[:], lhs[g][:, P * al:P * (al + 1)],
                        rhs[g][:, IC * be:IC * (be + 1)],
                        start=(g == 0), stop=(g == NT - 1),
                    )
                dst_ap = dst[al][:, IC * be:IC * (be + 1)]
                if be == 0:
                    nc.scalar.copy(dst_ap, mm[:])
                else:
                    nc.vector.tensor_scalar_add(dst_ap, mm[:], 0.0)
            if spill is not None:
                nc.sync.dma_start(spill[al][:], dst[al][:])

    square(Pf, Qf, Q2f, Q2d)
    square(Qf, Pf, P2f, None)
    square(P2f, Q2f, Q4f, Q4d)
    ph14.close()  # free the mm PSUM banks for the MLP pools

    h_ps = ctx.enter_context(tc.tile_pool(name="h_ps", bufs=2, space="PSUM"))
    o_ps = ctx.enter_context(tc.tile_pool(name="o_ps", bufs=2, space="PSUM"))

    # ---- edge MLP -----------------------------------------------------------
    for pi in range(NJB // 2):
        po = o_ps.tile([P, N], f32, tag="O")   # [(2jb x 8j x 8o), all i]
        for sub in range(2):
            jb = 2 * pi + sub
            seg = N * (jb % NSEG)
            if jb % 4 == 0:
                # fill 4 ring segments (4 j-blocks) per channel in one DMA:
                # dst [kk(8, partition), (jj c)(4096)]; src rows 8jb..8jb+32
                # of the spilled band, traversed kk-outer
                al = jb // 16
                r0 = JBLK * (jb % 16)
                base = N * (jb % NSEG)
                for s, srcd in enumerate((Qd, Q2d, Q4d)):
                    nc.gpsimd.dma_start(
                        ilbig[JBLK * s:JBLK * (s + 1), base:base + 4 * N],
                        srcd[al][r0:r0 + 4 * JBLK, :].rearrange(
                            "(jj kk) c -> kk jj c", kk=JBLK
                        ),
                    )
            h = h_ps.tile([P, N], f32, tag="H")
            for ic in range(NIC):
                nc.tensor.matmul(
                    h[:, IC * ic:IC * (ic + 1)], w1blk[:],
                    ilbig[:, seg + IC * ic:seg + IC * (ic + 1)],
                    start=True, stop=True,
                )
            nc.tensor.matmul(
                h[:, JBLK * jb:JBLK * jb + JBLK], w1diag[:], i8[:],
                start=False, stop=True, skip_group_check=True,
            )
            rt = rpool.tile([P, N], mm_dt, tag="R")
            if jb in VRELU:
                nc.vector.tensor_scalar(rt[:], h[:], b1rep[:], 0.0, add, amax)
            else:
                nc.scalar.activation(rt[:], h[:], relu_fn, bias=b1rep[:], scale=1.0)
            for ic in range(NIC):
                nc.tensor.matmul(
                    po[64 * sub:64 * (sub + 1), IC * ic:IC * (ic + 1)],
                    w2blk[:], rt[:, IC * ic:IC * (ic + 1)], start=True, stop=True,
                )
        ps = ppool.tile([P, N], mm_dt, tag="PS")
        nc.vector.tensor_scalar_add(ps[:], po[:], 0.0)
        nc.sync.dma_start(out[pi], ps[:])


def _build(mm_dtype_name="float16"):
    key = mm_dtype_name
    if key in _CACHE:
        return _CACHE[key]
    from contextlib import ExitStack
    import concourse.tile as tile
    from concourse import bacc, mybir

    nc = bacc.Bacc()
    with tile.TileContext(nc) as tc:
        with ExitStack() as ctx:
            _emit(nc, tc, ctx, getattr(mybir.dt, mm_dtype_name))
    nc.compile()
    _CACHE[key] = nc
    return nc


def _install_ntff_shim():
    """The agent image's antenv lacks axon_hooks; provide it and register the
    ctypes NTFF hook so run_bass_kernel_spmd(trace=True) can profile."""
    import sys
    import types

    if "antenv.axon_hooks" in sys.modules:
        return
    mod = types.ModuleType("antenv.axon_hooks")
    mod._hook = None
    mod.set_axon_ntff_profile_hook = lambda h: setattr(mod, "_hook", h)
    mod.get_axon_ntff_profile_hook = lambda: mod._hook
    sys.modules["antenv.axon_hooks"] = mod
    try:
        from trn_agent_boot.trn_boot import _ntff_profile_via_ctypes

        mod._hook = _ntff_profile_via_ctypes("/opt/axon/libaxon_pjrt.so")
    except Exception as e:  # degrade to no-trace
        print(f"ntff shim install failed: {e}")


def kernel(adj, mask, w1, b1, w2, b2):
    from concourse.bass_utils import run_bass_kernel_spmd

    global LAST_RESULTS
    adj = np.ascontiguousarray(np.asarray(adj, dtype=np.float32))
    mask = np.asarray(mask)
    w1 = np.ascontiguousarray(np.asarray(w1, dtype=np.float32))
    b1 = np.ascontiguousarray(np.asarray(b1, dtype=np.float32))
    w2 = np.ascontiguousarray(np.asarray(w2, dtype=np.float32))
    b2 = np.asarray(b2, dtype=np.float32)
    assert adj.shape == (B, N, N), adj.shape

    m = mask.astype(np.float32)
    general_mask = not np.all(m == 1.0)
    if general_mask:
        pair = m[:, :, None] * m[:, None, :]
        adj = np.ascontiguousarray(adj * pair)

    trace = bool(int(os.environ.get("KERNEL_TRACE", "0")))
    if trace:
        _install_ntff_shim()
    mmname = os.environ.get("KERNEL_MM_DT", "float16")
    nc = _build(mmname)

    from concourse import mybir

    np_mm = mybir.dt.np(getattr(mybir.dt, mmname))
    w1blk_np = np.zeros((3 * JBLK, P), np.float32)
    w1diag_np = np.zeros((JBLK, P), np.float32)
    w2blk_np = np.zeros((P, JBLK * HEADS), np.float32)
    for j in range(JBLK):
        for s in range(3):
            w1blk_np[JBLK * s + j, HID * j:HID * (j + 1)] = w1[s + 1]
        w1diag_np[j, HID * j:HID * (j + 1)] = w1[0]
        w2blk_np[HID * j:HID * (j + 1), HEADS * j:HEADS * (j + 1)] = w2
    shared = {
        "w1blk": w1blk_np.astype(np_mm),
        "w1diag": w1diag_np.astype(np_mm),
        "w2blk": w2blk_np.astype(np_mm),
        "b1rep": np.ascontiguousarray(np.tile(b1, JBLK).astype(np.float32)[:, None]),
        "i8": np.eye(JBLK, dtype=np_mm),
        "idn32": np.eye(P, dtype=np.float32),
    }
    in_maps = [{"adj": adj[c], **shared} for c in range(B)]
    res = run_bass_kernel_spmd(nc, in_maps, list(range(B)), trace=trace)
    LAST_RESULTS = res

    outs = []
    for c in range(B):
        o2 = np.asarray(res.results[c]["out"])          # [64, 128, 1024] fp16
        o2 = o2.reshape(NJB // 2, 2, JBLK, HEADS, N)    # [pi, sub, j', o, i]
        o2 = np.transpose(o2, (4, 0, 1, 2, 3))          # [i, pi, sub, j', o]
        outs.append(o2.reshape(N, N, HEADS).astype(np.float32))
    outp = np.stack(outs, axis=0)

    if np.any(b2 != 0.0):
        outp = outp + b2
    if general_mask:
        outp = outp * pair[..., None]
    return np.ascontiguousarray(outp.astype(np.float32))


# revision 8
# speedup vs baseline: 1.4422x; 1.4422x over previous
"""Trainium2 Bass kernel for nn_Diffuser_78331613544465.

Math (per graph b of B=8, N=1024):
    A   = adj (mask is all-ones in the graded setup; general mask handled host-side)
    P   = A / max(rowsum(A), 1)
    out[i,j,:] = relu([I, P, P2, P4][i,j,:] @ w1 + b1) @ w2 + b2   (P2=P@P, P4=P2@P2)

Device strategy: data-parallel over B — one graph per NeuronCore (8 cores).
On-chip work happens in the TRANSPOSED domain (Q = P^T).  Because A is
symmetric, both P = D^-1 A (row scale) and Q = A D^-1 (col scale) come from
cheap elementwise scalings of A, and the power chain needs NO PE transposes:
    Q2 = P^T Q,  P2 = Q^T P,  Q4 = P2^T Q2      (matmul(lhsT=X, rhs=Y) = X^T Y)

The edge MLP processes 32 j's at a time with the PE split into 16 32x32
tiles (tile_position): the staged rhs holds, per 32-row group r, the four
channels [I, Q, Q2, Q4] x 8 j's of block jb=4G+r interleaved as partition
32r+8s+jj; tile (r,c) applies a constant selector weight picking j-pair
(2c,2c+1) x 16 hidden.  The identity channel replaces the separate diagonal
matmul, and bias b1 rides the relu.  Layer 2 is the K=128 block-diagonal
matmul, M=64 col-group-paired.  The [16j x 8o, i] PSUM result is evacuated
as-is (fp16) and the HOST un-transposes — this keeps every output-DMA
descriptor a full 2KB partition line instead of 128B chunks.

The channels are staged via a DRAM interleave tensor IL4[N, 4, N] (identity
channel uploaded by the host, Q/Q2/Q4 spilled per 128-row band during the
power chain) so each 32-j group loads with ONE 256KB DMA.

kernel(**inputs) takes FULL inputs, shards over 8 cores, returns FULL output.
"""

import os
import numpy as np

B, N, P = 8, 1024, 128
HID, HEADS, NSTACK = 16, 8, 4
NT = N // P          # 8 row-tiles
JBLK = 8             # j rows per MLP block
NJB = N // JBLK      # 128 j-blocks
IC = 512             # i-chunk (matmul free dim)
NIC = N // IC        # 2
NGRP = N // 32       # 32 j-groups of 32 j's (4 j-blocks)

_CACHE = {}
LAST_RESULTS = None


def _emit(nc, tc, ctx, mm_dt):
    from concourse import mybir

    f32 = mybir.dt.float32
    add = mybir.AluOpType.add
    amax = mybir.AluOpType.max
    mult = mybir.AluOpType.mult
    relu_fn = mybir.ActivationFunctionType.Relu

    adj = nc.declare_dram_parameter("adj", [N, N], f32, isOutput=False)
    w1sel_d = nc.declare_dram_parameter("w1sel", [P, P], mm_dt, isOutput=False)
    w2blk_d = nc.declare_dram_parameter("w2blk", [P, JBLK * HEADS], mm_dt, isOutput=False)
    b1rep_d = nc.declare_dram_parameter("b1rep", [P, 1], f32, isOutput=False)
    idn32_d = nc.declare_dram_parameter("idn32", [P, P], f32, isOutput=False)
    idnil_d = nc.declare_dram_parameter("idnil", [N, N], mm_dt, isOutput=False)
    # device-natural output: [jb-pair, (16j x 8o) partition, i] in fp16;
    # host transposes to [i, j, o] and casts to f32
    out = nc.declare_dram_parameter("out", [NJB // 2, P, N], mm_dt, isOutput=True)

    from contextlib import ExitStack

    small = ctx.enter_context(tc.tile_pool(name="small", bufs=1))
    big = ctx.enter_context(tc.tile_pool(name="big", bufs=1))
    spool = ctx.enter_context(tc.tile_pool(name="spool", bufs=3))
    rpool = ctx.enter_context(tc.tile_pool(name="rpool", bufs=8))
    ppool = ctx.enter_context(tc.tile_pool(name="ppool", bufs=3))
    dram = ctx.enter_context(tc.tile_pool(name="dram", bufs=1, space="DRAM"))
    ph14 = ExitStack()
    mm_ps = ph14.enter_context(tc.tile_pool(name="mm_ps", bufs=2, space="PSUM"))

    # persistent matrices, one [128, 1024] tile per 128-row band
    Af = [big.tile([P, N], f32, name=f"Af{t}", tag=f"Af{t}") for t in range(NT)]
    Pf = [big.tile([P, N], mm_dt, name=f"Pf{t}", tag=f"Pf{t}") for t in range(NT)]
    Qf = [big.tile([P, N], mm_dt, name=f"Qf{t}", tag=f"Qf{t}") for t in range(NT)]
    Q2f = [big.tile([P, N], mm_dt, name=f"Q2f{t}", tag=f"Q2f{t}") for t in range(NT)]
    P2f = [big.tile([P, N], mm_dt, name=f"P2f{t}", tag=f"P2f{t}") for t in range(NT)]
    Q4f = [big.tile([P, N], mm_dt, name=f"Q4f{t}", tag=f"Q4f{t}") for t in range(NT)]
    invrep = big.tile([P, N], f32, tag="invrep")
    # DRAM channel-interleave [j, s, i]: s=0 identity (host), 1..3 = Q,Q2,Q4
    il4 = dram.tile([N, NSTACK, N], mm_dt, tag="il4")

    # ---- constants / weights (host-prepared; one DMA each) -----------------
    idn32 = small.tile([P, P], f32, tag="idn32")
    nc.gpsimd.dma_start(idn32[:], idn32_d[:])
    ones1 = small.tile([1, P], f32, tag="ones1")
    nc.vector.memset(ones1[:], 1.0)
    w1sel = small.tile([P, P], mm_dt, tag="w1sel")
    nc.gpsimd.dma_start(w1sel[:], w1sel_d[:])
    w2blk = small.tile([P, JBLK * HEADS], mm_dt, tag="w2blk")
    nc.gpsimd.dma_start(w2blk[:], w2blk_d[:])
    b1rep = small.tile([P, 1], f32, tag="b1rep")
    nc.gpsimd.dma_start(b1rep[:], b1rep_d[:])
    # identity channel of the interleave (DRAM -> DRAM, once)
    nc.sync.dma_start(il4[:, 0:1, :], idnil_d[:])

    # ---- phase 1: load adj, deg -> invdeg, P = rowscale(A) ------------------
    invcol = small.tile([P, NT], f32, tag="invcol")
    for t in range(NT):
        nc.gpsimd.dma_start(Af[t][:], adj[P * t:P * (t + 1), :])
        deg = small.tile([P, 1], f32, tag=f"deg{t}")
        nc.vector.tensor_reduce(
            deg[:], Af[t][:], axis=mybir.AxisListType.X, op=add,
        )
        degc = small.tile([P, 1], f32, tag=f"degc{t}")
        nc.vector.tensor_scalar_max(degc[:], deg[:], 1.0)
        nc.vector.reciprocal(invcol[:, t:t + 1], degc[:])
        # P = A * invdeg[row]  (per-partition scale on the scalar engine)
        nc.scalar.mul(Pf[t][:], Af[t][:], invcol[:, t:t + 1])

    # invrep[p, c] = invdeg(row c) for all p  (transpose + broadcast via PE)
    invrow = small.tile([1, N], f32, tag="invrow")
    for t in range(NT):
        ptp = mm_ps.tile([P, P], f32, tag="pt")
        nc.tensor.transpose(ptp[0:1, :], invcol[:, t:t + 1], idn32[:])
        nc.scalar.copy(invrow[0:1, P * t:P * (t + 1)], ptp[0:1, :])
    for half in range(2):
        pb = mm_ps.tile([P, IC], f32, tag="mm")
        for k in range(4):
            c = 4 * half + k
            nc.tensor.matmul(
                pb[:, P * k:P * (k + 1)], ones1[:], invrow[0:1, P * c:P * (c + 1)],
                start=True, stop=True,
            )
        nc.scalar.copy(invrep[:, IC * half:IC * (half + 1)], pb[:])

    # Q = A * invdeg[col]; spill each band into the interleave
    for t in range(NT):
        eng = nc.vector if t % 2 == 0 else nc.gpsimd
        eng.tensor_tensor(Qf[t][:], Af[t][:], invrep[:], op=mult)
        nc.sync.dma_start(il4[P * t:P * (t + 1), 1:2, :], Qf[t][:])

    # ---- power chain: X2[r,c] = sum_k L[k,r] R[k,c]  (no transposes) --------
    def square(lhs, rhs, dst, chan):
        for al in range(NT):
            mm = mm_ps.tile([P, N], f32, tag="mm")
            for be in range(NIC):
                for g in range(NT):
                    nc.tensor.matmul(
                        mm[:, IC * be:IC * (be + 1)],
                        lhs[g][:, P * al:P * (al + 1)],
                        rhs[g][:, IC * be:IC * (be + 1)],
                        start=(g == 0), stop=(g == NT - 1),
                    )
            if al % 2 == 0:
                nc.scalar.copy(dst[al][:], mm[:])
            else:
                nc.vector.tensor_scalar_add(dst[al][:], mm[:], 0.0)
            if chan is not None:
                nc.sync.dma_start(il4[P * al:P * (al + 1), chan:chan + 1, :], dst[al][:])

    square(Pf, Qf, Q2f, 2)
    square(Qf, Pf, P2f, None)
    square(P2f, Q2f, Q4f, 3)
    ph14.close()  # free the mm PSUM banks for the MLP pools

    h_ps = ctx.enter_context(tc.tile_pool(name="h_ps", bufs=6, space="PSUM"))
    o_ps = ctx.enter_context(tc.tile_pool(name="o_ps", bufs=1, space="PSUM"))

    # ---- edge MLP: 32 j's per group, PE as 16 32x32 tiles -------------------
    for G in range(NGRP):
        stage = spool.tile([P, N], mm_dt, tag="S")
        # stage 4 channels x 8 j's per row group: partition 32r+8s+jj
        for r in range(4):
            j0 = 32 * G + 8 * r
            nc.gpsimd.dma_start(
                stage[32 * r:32 * (r + 1), :],
                il4[j0:j0 + 8, :, :].rearrange("jj s c -> s jj c"),
            )
        hs = {}
        for ic in range(NIC):
            for r in range(4):
                h = h_ps.tile([P, IC], f32, tag="H")
                hs[(r, ic)] = h
                for c in range(4):
                    nc.tensor.matmul(
                        h[32 * c:32 * (c + 1), :],
                        w1sel[32 * r:32 * (r + 1), 32 * c:32 * (c + 1)],
                        stage[32 * r:32 * (r + 1), IC * ic:IC * (ic + 1)],
                        start=True, stop=True, tile_position=(32 * r, 32 * c),
                    )
        rts = {}
        for r in range(4):
            for ic in range(NIC):
                rt = rpool.tile([P, IC], mm_dt, tag="R")
                rts[(r, ic)] = rt
                h = hs[(r, ic)]
                if (8 * G + 2 * r + ic) % 9 in (4, 8):
                    nc.vector.tensor_scalar(rt[:], h[:], b1rep[:], 0.0, add, amax)
                else:
                    nc.scalar.activation(rt[:], h[:], relu_fn, bias=b1rep[:], scale=1.0)
        for half in range(2):
            po = o_ps.tile([P, N], f32, tag="O")
            for sub in range(2):
                for ic in range(NIC):
                    nc.tensor.matmul(
                        po[64 * sub:64 * (sub + 1), IC * ic:IC * (ic + 1)],
                        w2blk[:], rts[(2 * half + sub, ic)][:],
                        start=True, stop=True,
                    )
            ps = ppool.tile([P, N], mm_dt, tag="PS")
            nc.vector.tensor_scalar_add(ps[:], po[:], 0.0)
            nc.sync.dma_start(out[2 * G + half], ps[:])


def _build(mm_dtype_name="float16"):
    key = mm_dtype_name
    if key in _CACHE:
        return _CACHE[key]
    from contextlib import ExitStack
    import concourse.tile as tile
    from concourse import bacc, mybir

    nc = bacc.Bacc()
    with tile.TileContext(nc) as tc:
        with ExitStack() as ctx:
            _emit(nc, tc, ctx, getattr(mybir.dt, mm_dtype_name))
    nc.compile()
    _CACHE[key] = nc
    return nc


def _install_ntff_shim():
    """The agent image's antenv lacks axon_hooks; provide it and register the
    ctypes NTFF hook so run_bass_kernel_spmd(trace=True) can profile."""
    import sys
    import types

    if "antenv.axon_hooks" in sys.modules:
        return
    mod = types.ModuleType("antenv.axon_hooks")
    mod._hook = None
    mod.set_axon_ntff_profile_hook = lambda h: setattr(mod, "_hook", h)
    mod.get_axon_ntff_profile_hook = lambda: mod._hook
    sys.modules["antenv.axon_hooks"] = mod
    try:
        from trn_agent_boot.trn_boot import _ntff_profile_via_ctypes

        mod._hook = _ntff_profile_via_ctypes("/opt/axon/libaxon_pjrt.so")
    except Exception as e:  # degrade to no-trace
        print(f"ntff shim install failed: {e}")


def _host_tensors(w1, b1, w2, np_mm):
    # selector weights: tile (r,c) maps staged rows (s, jj) -> (jj', hid) of
    # j-pair (2c, 2c+1); identical for all four row groups r
    w1sel_np = np.zeros((P, P), np.float32)
    for r in range(4):
        for s in range(NSTACK):
            for c in range(4):
                for jj in range(2):
                    j = 2 * c + jj
                    w1sel_np[32 * r + JBLK * s + j,
                             32 * c + HID * jj:32 * c + HID * (jj + 1)] = w1[s]
    w2blk_np = np.zeros((P, JBLK * HEADS), np.float32)
    for j in range(JBLK):
        w2blk_np[HID * j:HID * (j + 1), HEADS * j:HEADS * (j + 1)] = w2
    return {
        "w1sel": w1sel_np.astype(np_mm),
        "w2blk": w2blk_np.astype(np_mm),
        "b1rep": np.ascontiguousarray(np.tile(b1, JBLK).astype(np.float32)[:, None]),
        "idn32": np.eye(P, dtype=np.float32),
        "idnil": np.eye(N, dtype=np_mm),
    }


def kernel(adj, mask, w1, b1, w2, b2):
    from concourse.bass_utils import run_bass_kernel_spmd

    global LAST_RESULTS
    adj = np.ascontiguousarray(np.asarray(adj, dtype=np.float32))
    mask = np.asarray(mask)
    w1 = np.ascontiguousarray(np.asarray(w1, dtype=np.float32))
    b1 = np.ascontiguousarray(np.asarray(b1, dtype=np.float32))
    w2 = np.ascontiguousarray(np.asarray(w2, dtype=np.float32))
    b2 = np.asarray(b2, dtype=np.float32)
    assert adj.shape == (B, N, N), adj.shape

    m = mask.astype(np.float32)
    general_mask = not np.all(m == 1.0)
    if general_mask:
        pair = m[:, :, None] * m[:, None, :]
        adj = np.ascontiguousarray(adj * pair)

    trace = bool(int(os.environ.get("KERNEL_TRACE", "0")))
    if trace:
        _install_ntff_shim()
    mmname = os.environ.get("KERNEL_MM_DT", "float16")
    nc = _build(mmname)

    from concourse import mybir

    np_mm = mybir.dt.np(getattr(mybir.dt, mmname))
    shared = _host_tensors(w1, b1, w2, np_mm)
    in_maps = [{"adj": adj[c], **shared} for c in range(B)]
    res = run_bass_kernel_spmd(nc, in_maps, list(range(B)), trace=trace)
    LAST_RESULTS = res

    outs = []
    for c in range(B):
        o2 = np.asarray(res.results[c]["out"])          # [64, 128, 1024] fp16
        o2 = o2.reshape(NJB // 2, 2, JBLK, HEADS, N)    # [pi, sub, j', o, i]
        o2 = np.transpose(o2, (4, 0, 1, 2, 3))          # [i, pi, sub, j', o]
        outs.append(o2.reshape(N, N, HEADS).astype(np.float32))
    outp = np.stack(outs, axis=0)

    if np.any(b2 != 0.0):
        outp = outp + b2
    if general_mask:
        outp = outp * pair[..., None]
    return np.ascontiguousarray(outp.astype(np.float32))


# revision 13
# speedup vs baseline: 1.5291x; 1.0602x over previous
"""Trainium2 Bass kernel for nn_Diffuser_78331613544465.

Math (per graph b of B=8, N=1024):
    A   = adj (mask is all-ones in the graded setup; general mask handled host-side)
    P   = A / max(rowsum(A), 1)
    out[i,j,:] = relu([I, P, P2, P4][i,j,:] @ w1 + b1) @ w2 + b2   (P2=P@P, P4=P2@P2)

Device strategy: data-parallel over B — one graph per NeuronCore (8 cores).
On-chip work happens in the TRANSPOSED domain (Q = P^T).  Because A is
symmetric, both P = D^-1 A (row scale) and Q = A D^-1 (col scale) come from
cheap elementwise scalings of A, and the power chain needs NO PE transposes:
    Q2 = P^T Q,  P2 = Q^T P,  Q4 = P2^T Q2      (matmul(lhsT=X, rhs=Y) = X^T Y)

The edge MLP processes 32 j's at a time with the PE split into 16 32x32
tiles (tile_position): the staged rhs holds, per 32-row group r, the four
channels [I, Q, Q2, Q4] x 8 j's of block jb=4G+r interleaved as partition
32r+8s+jj; tile (r,c) applies a constant selector weight picking j-pair
(2c,2c+1) x 16 hidden.  The identity channel replaces the separate diagonal
matmul, and bias b1 rides the relu.  Layer 2 is the K=128 block-diagonal
matmul, M=64 col-group-paired.  The [16j x 8o, i] PSUM result is evacuated
as-is (fp16) and the HOST un-transposes — this keeps every output-DMA
descriptor a full 2KB partition line instead of 128B chunks.

The channels are staged via a DRAM interleave tensor IL4[N, 4, N] (identity
channel uploaded by the host, Q/Q2/Q4 spilled per 128-row band during the
power chain) so each 32-j group loads with ONE 256KB DMA.

kernel(**inputs) takes FULL inputs, shards over 8 cores, returns FULL output.
"""

import os
import numpy as np

B, N, P = 8, 1024, 128
HID, HEADS, NSTACK = 16, 8, 4
NT = N // P          # 8 row-tiles
JBLK = 8             # j rows per MLP block
NJB = N // JBLK      # 128 j-blocks
IC = 512             # i-chunk (matmul free dim)
NIC = N // IC        # 2
NGRP = N // 32       # 32 j-groups of 32 j's (4 j-blocks)

_CACHE = {}
LAST_RESULTS = None


def _emit(nc, tc, ctx, mm_dt):
    from concourse import mybir

    f32 = mybir.dt.float32
    add = mybir.AluOpType.add
    amax = mybir.AluOpType.max
    mult = mybir.AluOpType.mult
    relu_fn = mybir.ActivationFunctionType.Relu

    adj = nc.declare_dram_parameter("adj", [N, N], f32, isOutput=False)
    w1sel_d = nc.declare_dram_parameter("w1sel", [P, P], mm_dt, isOutput=False)
    w2blk_d = nc.declare_dram_parameter("w2blk", [P, JBLK * HEADS], mm_dt, isOutput=False)
    b1rep_d = nc.declare_dram_parameter("b1rep", [P, 1], f32, isOutput=False)
    idn32_d = nc.declare_dram_parameter("idn32", [P, P], f32, isOutput=False)
    idnil_d = nc.declare_dram_parameter("idnil", [N, N], mm_dt, isOutput=False)
    # device-natural output: [jb-pair, (16j x 8o) partition, i] in fp16;
    # host transposes to [i, j, o] and casts to f32
    out = nc.declare_dram_parameter("out", [NJB // 2, P, N], mm_dt, isOutput=True)

    from contextlib import ExitStack

    small = ctx.enter_context(tc.tile_pool(name="small", bufs=1))
    big = ctx.enter_context(tc.tile_pool(name="big", bufs=1))
    spool = ctx.enter_context(tc.tile_pool(name="spool", bufs=3))
    rpool = ctx.enter_context(tc.tile_pool(name="rpool", bufs=8))
    ppool = ctx.enter_context(tc.tile_pool(name="ppool", bufs=3))
    dram = ctx.enter_context(tc.tile_pool(name="dram", bufs=1, space="DRAM"))
    ph14 = ExitStack()
    mm_ps = ph14.enter_context(tc.tile_pool(name="mm_ps", bufs=2, space="PSUM"))

    # persistent matrices, one [128, 1024] tile per 128-row band
    Af = [big.tile([P, N], mm_dt, name=f"Af{t}", tag=f"Af{t}") for t in range(NT)]
    Pf = [big.tile([P, N], mm_dt, name=f"Pf{t}", tag=f"Pf{t}") for t in range(NT)]
    Qf = [big.tile([P, N], mm_dt, name=f"Qf{t}", tag=f"Qf{t}") for t in range(NT)]
    Q2f = [big.tile([P, N], mm_dt, name=f"Q2f{t}", tag=f"Q2f{t}") for t in range(NT)]
    P2f = [big.tile([P, N], mm_dt, name=f"P2f{t}", tag=f"P2f{t}") for t in range(NT)]
    Q4f = [big.tile([P, N], mm_dt, name=f"Q4f{t}", tag=f"Q4f{t}") for t in range(NT)]
    invrep = big.tile([P, N], f32, tag="invrep")
    # DRAM channel-interleave [j, s, i]: s=0 identity (host), 1..3 = Q,Q2,Q4
    il4 = dram.tile([N, NSTACK, N], mm_dt, tag="il4")

    # ---- constants / weights (host-prepared; one DMA each) -----------------
    idn32 = small.tile([P, P], f32, tag="idn32")
    nc.gpsimd.dma_start(idn32[:], idn32_d[:])
    ones1 = small.tile([1, P], f32, tag="ones1")
    nc.vector.memset(ones1[:], 1.0)
    w1sel = small.tile([P, P], mm_dt, tag="w1sel")
    nc.gpsimd.dma_start(w1sel[:], w1sel_d[:])
    w2blk = small.tile([P, JBLK * HEADS], mm_dt, tag="w2blk")
    nc.gpsimd.dma_start(w2blk[:], w2blk_d[:])
    b1rep = small.tile([P, 1], f32, tag="b1rep")
    nc.gpsimd.dma_start(b1rep[:], b1rep_d[:])
    # identity channel of the interleave (DRAM -> DRAM, once)
    nc.sync.dma_start(il4[:, 0:1, :], idnil_d[:])

    # ---- phase 1: load adj (fp16 via DMA cast), deg -> invdeg, P ------------
    invcol = small.tile([P, NT], f32, tag="invcol")
    for t in range(NT):
        nc.gpsimd.dma_start(Af[t][:], adj[P * t:P * (t + 1), :])
        deg = small.tile([P, 1], f32, tag=f"deg{t}")
        nc.vector.tensor_reduce(
            deg[:], Af[t][:], axis=mybir.AxisListType.X, op=add,
        )
        degc = small.tile([P, 1], f32, tag=f"degc{t}")
        nc.vector.tensor_scalar_max(degc[:], deg[:], 1.0)
        nc.vector.reciprocal(invcol[:, t:t + 1], degc[:])
        # P = A * invdeg[row]  (per-partition scale on the scalar engine)
        nc.scalar.mul(Pf[t][:], Af[t][:], invcol[:, t:t + 1])

    # invrep[p, c] = invdeg(row c) for all p  (transpose + broadcast via PE)
    invrow = small.tile([1, N], f32, tag="invrow")
    for t in range(NT):
        ptp = mm_ps.tile([P, P], f32, tag="pt")
        nc.tensor.transpose(ptp[0:1, :], invcol[:, t:t + 1], idn32[:])
        nc.scalar.copy(invrow[0:1, P * t:P * (t + 1)], ptp[0:1, :])
    for half in range(2):
        pb = mm_ps.tile([P, IC], f32, tag="mm")
        for k in range(4):
            c = 4 * half + k
            nc.tensor.matmul(
                pb[:, P * k:P * (k + 1)], ones1[:], invrow[0:1, P * c:P * (c + 1)],
                start=True, stop=True,
            )
        nc.scalar.copy(invrep[:, IC * half:IC * (half + 1)], pb[:])

    # Q = A * invdeg[col]; spill each band into the interleave
    for t in range(NT):
        eng = nc.vector if t % 2 == 0 else nc.gpsimd
        eng.tensor_tensor(Qf[t][:], Af[t][:], invrep[:], op=mult)
        nc.sync.dma_start(il4[P * t:P * (t + 1), 1:2, :], Qf[t][:])

    # ---- power chain (no transposes; M3 := A D^-1 A is symmetric, so ONE
    # square yields both Q2 = M3 D^-1 (col scale) and P2 = D^-1 M3 (row
    # scale); then Q4 = Q2^2 = P2^T Q2) -------------------------------------
    for al in range(NT):
        mm = mm_ps.tile([P, N], f32, tag="mm")
        for be in range(NIC):
            for g in range(NT):
                nc.tensor.matmul(
                    mm[:, IC * be:IC * (be + 1)],
                    Af[g][:, P * al:P * (al + 1)],
                    Pf[g][:, IC * be:IC * (be + 1)],
                    start=(g == 0), stop=(g == NT - 1),
                )
        nc.vector.tensor_tensor(Q2f[al][:], mm[:], invrep[:], op=mult)
        nc.scalar.mul(P2f[al][:], mm[:], invcol[:, al:al + 1])
        nc.sync.dma_start(il4[P * al:P * (al + 1), 2:3, :], Q2f[al][:])
    for al in range(NT):
        mm = mm_ps.tile([P, N], f32, tag="mm")
        for be in range(NIC):
            for g in range(NT):
                nc.tensor.matmul(
                    mm[:, IC * be:IC * (be + 1)],
                    P2f[g][:, P * al:P * (al + 1)],
                    Q2f[g][:, IC * be:IC * (be + 1)],
                    start=(g == 0), stop=(g == NT - 1),
                )
        if al % 2 == 0:
            nc.scalar.copy(Q4f[al][:], mm[:])
        else:
            nc.vector.tensor_scalar_add(Q4f[al][:], mm[:], 0.0)
        nc.sync.dma_start(il4[P * al:P * (al + 1), 3:4, :], Q4f[al][:])
    ph14.close()  # free the mm PSUM banks for the MLP pools

    h_ps = ctx.enter_context(tc.tile_pool(name="h_ps", bufs=4, space="PSUM"))
    o_ps = ctx.enter_context(tc.tile_pool(name="o_ps", bufs=2, space="PSUM"))

    # ---- edge MLP: 32 j's per group, PE as 16 32x32 tiles -------------------
    for G in range(NGRP):
        stage = spool.tile([P, N], mm_dt, tag="S")
        # stage 4 channels x 8 j's per row group: partition 32r+8s+jj
        for r in range(4):
            j0 = 32 * G + 8 * r
            nc.gpsimd.dma_start(
                stage[32 * r:32 * (r + 1), :],
                il4[j0:j0 + 8, :, :].rearrange("jj s c -> s jj c"),
            )
        hs = {}
        for ic in range(NIC):
            for r in range(4):
                h = h_ps.tile([P, IC], f32, tag="H")
                hs[(r, ic)] = h
                for c in range(4):
                    nc.tensor.matmul(
                        h[32 * c:32 * (c + 1), :],
                        w1sel[32 * r:32 * (r + 1), 32 * c:32 * (c + 1)],
                        stage[32 * r:32 * (r + 1), IC * ic:IC * (ic + 1)],
                        start=True, stop=True, tile_position=(32 * r, 32 * c),
                    )
        rts = {}
        for ic in range(NIC):
            for r in range(4):
                rt = rpool.tile([P, IC], mm_dt, tag="R")
                rts[(r, ic)] = rt
                h = hs[(r, ic)]
                if (8 * G + 4 * ic + r) % 17 < 8:
                    nc.vector.tensor_scalar(rt[:], h[:], b1rep[:], 0.0, add, amax)
                else:
                    nc.scalar.activation(rt[:], h[:], relu_fn, bias=b1rep[:], scale=1.0)
        for half in range(2):
            po = o_ps.tile([P, N], f32, tag="O")
            for sub in range(2):
                for ic in range(NIC):
                    nc.tensor.matmul(
                        po[64 * sub:64 * (sub + 1), IC * ic:IC * (ic + 1)],
                        w2blk[:], rts[(2 * half + sub, ic)][:],
                        start=True, stop=True,
                    )
            ps = ppool.tile([P, N], mm_dt, tag="PS")
            if (2 * G + half) % 2 == 0:
                nc.vector.tensor_scalar_add(ps[:], po[:], 0.0)
            else:
                nc.scalar.copy(ps[:], po[:])
            nc.sync.dma_start(out[2 * G + half], ps[:])


def _build(mm_dtype_name="float16"):
    key = mm_dtype_name
    if key in _CACHE:
        return _CACHE[key]
    from contextlib import ExitStack
    import concourse.tile as tile
    from concourse import bacc, mybir

    nc = bacc.Bacc()
    with tile.TileContext(nc) as tc:
        with ExitStack() as ctx:
            _emit(nc, tc, ctx, getattr(mybir.dt, mm_dtype_name))
    nc.compile()
    _CACHE[key] = nc
    return nc


def _install_ntff_shim():
    """The agent image's antenv lacks axon_hooks; provide it and register the
    ctypes NTFF hook so run_bass_kernel_spmd(trace=True) can profile."""
    import sys
    import types

    if "antenv.axon_hooks" in sys.modules:
        return
    mod = types.ModuleType("antenv.axon_hooks")
    mod._hook = None
    mod.set_axon_ntff_profile_hook = lambda h: setattr(mod, "_hook", h)
    mod.get_axon_ntff_profile_hook = lambda: mod._hook
    sys.modules["antenv.axon_hooks"] = mod
    try:
        from trn_agent_boot.trn_boot import _ntff_profile_via_ctypes

        mod._hook = _ntff_profile_via_ctypes("/opt/axon/libaxon_pjrt.so")
    except Exception as e:  # degrade to no-trace
        print(f"ntff shim install failed: {e}")


def _host_tensors(w1, b1, w2, np_mm):
    # selector weights: tile (r,c) maps staged rows (s, jj) -> (jj', hid) of
    # j-pair (2c, 2c+1); identical for all four row groups r
    w1sel_np = np.zeros((P, P), np.float32)
    for r in range(4):
        for s in range(NSTACK):
            for c in range(4):
                for jj in range(2):
                    j = 2 * c + jj
                    w1sel_np[32 * r + JBLK * s + j,
                             32 * c + HID * jj:32 * c + HID * (jj + 1)] = w1[s]
    w2blk_np = np.zeros((P, JBLK * HEADS), np.float32)
    for j in range(JBLK):
        w2blk_np[HID * j:HID * (j + 1), HEADS * j:HEADS * (j + 1)] = w2
    return {
        "w1sel": w1sel_np.astype(np_mm),
        "w2blk": w2blk_np.astype(np_mm),
        "b1rep": np.ascontiguousarray(np.tile(b1, JBLK).astype(np.float32)[:, None]),
        "idn32": np.eye(P, dtype=np.float32),
        "idnil": np.eye(N, dtype=np_mm),
    }


def kernel(adj, mask, w1, b1, w2, b2):
    from concourse.bass_utils import run_bass_kernel_spmd

    global LAST_RESULTS
    adj = np.ascontiguousarray(np.asarray(adj, dtype=np.float32))
    mask = np.asarray(mask)
    w1 = np.ascontiguousarray(np.asarray(w1, dtype=np.float32))
    b1 = np.ascontiguousarray(np.asarray(b1, dtype=np.float32))
    w2 = np.ascontiguousarray(np.asarray(w2, dtype=np.float32))
    b2 = np.asarray(b2, dtype=np.float32)
    assert adj.shape == (B, N, N), adj.shape

    m = mask.astype(np.float32)
    general_mask = not np.all(m == 1.0)
    if general_mask:
        pair = m[:, :, None] * m[:, None, :]
        adj = np.ascontiguousarray(adj * pair)

    trace = bool(int(os.environ.get("KERNEL_TRACE", "0")))
    if trace:
        _install_ntff_shim()
    mmname = os.environ.get("KERNEL_MM_DT", "float16")
    nc = _build(mmname)

    from concourse import mybir

    np_mm = mybir.dt.np(getattr(mybir.dt, mmname))
    shared = _host_tensors(w1, b1, w2, np_mm)
    in_maps = [{"adj": adj[c], **shared} for c in range(B)]
    res = run_bass_kernel_spmd(nc, in_maps, list(range(B)), trace=trace)
    LAST_RESULTS = res

    outs = []
    for c in range(B):
        o2 = np.asarray(res.results[c]["out"])          # [64, 128, 1024] fp16
        o2 = o2.reshape(NJB // 2, 2, JBLK, HEADS, N)    # [pi, sub, j', o, i]
        o2 = np.transpose(o2, (4, 0, 1, 2, 3))          # [i, pi, sub, j', o]
        outs.append(o2.reshape(N, N, HEADS).astype(np.float32))
    outp = np.stack(outs, axis=0)

    if np.any(b2 != 0.0):
        outp = outp + b2
    if general_mask:
        outp = outp * pair[..., None]
    return np.ascontiguousarray(outp.astype(np.float32))
